# revision 4
# baseline (speedup 1.0000x reference)
"""Trainium2 Bass kernel for GTPCA topk_masking layer.

Computation (see reference):
  wn     = w / sqrt(sum(w^2)/n),  n = 128*128
  scores = valid_xcorr2d(inputs, wn) / n          -> (B, 113, 113)
  idx    = argmax |scores| (flat, first occurrence)
  out    = scores[idx] * wn placed as a 16x16 patch at idx, zeros elsewhere

Device strategy (pure data parallel over 8 cores, 512 images each):
  - The 2D correlation runs on the tensor engine as 8 accumulating fp8
    DoubleRow matmuls per 4-image bank.  DoubleRow virtualizes the
    contraction to 256 rows (2 fp8 weights per PE cell, 2 MACs/cycle), so
    kernel columns are processed in pairs (2j, 2j+1): the stationary holds
    the two 128x128 Toeplitz matrices T_{2j} / T_{2j+1} and the moving
    operand holds two column-shifted copies of the image rows (packed by
    the host as separate slabs so no overlapping access patterns are
    needed).  This streams 912 moving columns per image instead of the
    1824 an fp32r kernel needs -> ~2x tensor-engine throughput.
  - Per PSUM bank (4 images) one fused DVE reduce with
    apply_absolute_value produces the per-row abs-max of the score map.
  - Only the per-row abs-max [113, 512] leaves the device.  The host
    keeps candidate rows within CAND_TOL of each image's device global
    max (the gate is sized for fp8 quantization noise), rescores those
    rows exactly in fp64 from the original fp32 inputs, picks the true
    argmax, and scatters smax*wn patches into the output.
"""

import sys

import numpy as np

if "/opt/trn_rl_repo" not in sys.path:
    sys.path.insert(0, "/opt/trn_rl_repo")

import ml_dtypes

F8 = ml_dtypes.float8_e4m3  # TRN FP8_EXP4 (bias 7, max +-240)

N_CORES = 8
B = 4096
H = W = 128
KH = KW = 16
SH = SW = H - KH + 1  # 113
SW_PAD = 114  # streamed score columns per q-pair (even count required; col
# 113 is garbage, excluded from the reduce)
W2 = 130  # per-slab padded image width (c reads reach col 127; slab ko=1
# col 127 on q-pair 7 is the zero pad; 130 keeps the quad run 16B-aligned)
N_ELEM = H * W  # 16384
PER_CORE = B // N_CORES  # 512
GROUP = 16  # images per DMA/compute group
BANK = 4  # images per PSUM bank (456 fp32 <= 512 psum limit)
QP = KW // 2  # 8 q-pairs (DoubleRow processes 2 kernel columns per pass)
CAND_TOL = 8e-2  # candidate-row gate vs device global max (fp8 noise margin)


def _build_nc(n_imgs: int, repeat: int = 1):
    from contextlib import ExitStack

    import concourse.bacc as bacc
    import concourse.mybir as mybir
    import concourse.tile as tile

    f32 = mybir.dt.float32
    f8 = mybir.dt.float8e4

    nc = bacc.Bacc("TRN2", target_bir_lowering=False)
    # imgs[h, quad, ko, c, i] = fp8(img_{4*quad+i}[h, c+ko])  (0 out of range)
    imgs_d = nc.dram_tensor(
        "imgs", [H, n_imgs // BANK, 2, W2, BANK], f8, kind="ExternalInput"
    )
    # ttoe[h, j, ko, m] = fp8(wn[h-m, 2j+ko]) for 0<=h-m<16, m<=112; else 0
    ttoe_d = nc.dram_tensor("ttoe", [H, QP, 2, H], f8, kind="ExternalInput")
    rm_d = nc.dram_tensor("rowmax", [SH, n_imgs], f32, kind="ExternalOutput")

    n_groups = n_imgs // GROUP
    banks_per_group = GROUP // BANK

    with ExitStack() as ctx:
        tc = ctx.enter_context(tile.TileContext(nc))
        consts = ctx.enter_context(tc.tile_pool(name="consts", bufs=1))
        imgp = ctx.enter_context(tc.tile_pool(name="imgp", bufs=3))
        accp = ctx.enter_context(tc.tile_pool(name="accp", bufs=2, space="PSUM"))
        stage = ctx.enter_context(tc.tile_pool(name="stage", bufs=1))

        ttoe_t = consts.tile([H, QP, 2, H], f8)
        nc.sync.dma_start(ttoe_t[:], ttoe_d[:])
        rm_all = stage.tile([SH, n_imgs], f32)

        for _rep in range(repeat):
          for g in range(n_groups):
            img_t = imgp.tile([H, banks_per_group, 2, W2, BANK], f8)
            nc.sync.dma_start(
                img_t[:],
                imgs_d[:, g * banks_per_group : (g + 1) * banks_per_group],
            )

            psums = [
                accp.tile(
                    [H, SW_PAD, BANK], f32, name=f"acc{bk}", tag=f"acc{bk}"
                )
                for bk in range(banks_per_group)
            ]
            for j in range(QP):
                lhsT = ttoe_t[:, j]  # [128, 2, 128]
                for bk in range(banks_per_group):
                    # [128, 2, 114, 4]; (c, i) dims merge to one 456-run
                    rhs = img_t[:, bk, :, 2 * j : 2 * j + SW_PAD, :]
                    nc.tensor.matmul(
                        psums[bk][:],
                        lhsT,
                        rhs,
                        start=(j == 0),
                        stop=(j == QP - 1),
                        perf_mode=mybir.MatmulPerfMode.DoubleRow,
                        skip_group_check=True,
                    )
            for bk in range(banks_per_group):
                base = g * GROUP + bk * BANK
                # [113, 113, 4] -> dims (part, img, c) so X-reduce is over c
                red_in = psums[bk][0:SH, 0:SW, :].transpose([0, 2, 1])
                nc.vector.tensor_reduce(
                    rm_all[:, base : base + BANK],
                    red_in,
                    axis=mybir.AxisListType.X,
                    op=mybir.AluOpType.max,
                    apply_absolute_value=True,
                )

        nc.sync.dma_start(rm_d[:], rm_all[:])

    nc.compile()
    return nc


_NC_CACHE: dict = {}


def _get_nc(n_imgs: int):
    if n_imgs not in _NC_CACHE:
        _NC_CACHE[n_imgs] = _build_nc(n_imgs)
    return _NC_CACHE[n_imgs]


def _weights_f32(w: np.ndarray) -> np.ndarray:
    w32 = np.asarray(w, dtype=np.float32)
    ss = np.sum(w32 * w32, dtype=np.float32)
    denom = np.sqrt(ss / np.float32(N_ELEM))
    return (w32 / denom).astype(np.float32)


def _pack_ttoe(wn: np.ndarray) -> np.ndarray:
    """fp8 stationary: [H, QP, 2, H]; [:, j, ko, m] = T_{2j+ko}[:, m]."""
    wn8 = wn.astype(F8).astype(np.float32)
    ttoe = np.zeros((H, QP, 2, H), dtype=np.float32)
    for m in range(SH):
        for j in range(QP):
            ttoe[m : m + KH, j, 0, m] = wn8[:, 2 * j]
            ttoe[m : m + KH, j, 1, m] = wn8[:, 2 * j + 1]
    return ttoe.astype(F8)


def _pack_imgs(shard: np.ndarray) -> np.ndarray:
    """fp8 moving operand for one core's shard [n, 128, 128] (f32).

    Returns [H, n//4, 2, W2, 4]: two column-shifted fp8 slabs per 4-image
    quad so DoubleRow's ko dim reads non-overlapping memory.
    """
    n = shard.shape[0]
    a8 = np.ascontiguousarray(shard.transpose(1, 0, 2)).astype(F8)  # [H, n, W]
    v = a8.reshape(H, n // BANK, BANK, W).transpose(0, 1, 3, 2)  # [H, q, c, i]
    out = np.zeros((H, n // BANK, 2, W2, BANK), dtype=F8)
    out[:, :, 0, 0:W, :] = v
    out[:, :, 1, 0 : W - 1, :] = v[:, :, 1:, :]
    return out


def _run_device(inputs_np: np.ndarray, wn: np.ndarray, trace: bool = False):
    from concourse.bass_utils import run_bass_kernel_spmd

    nc = _get_nc(PER_CORE)
    ttoe = _pack_ttoe(wn)
    in_maps = []
    for c in range(N_CORES):
        shard = inputs_np[c * PER_CORE : (c + 1) * PER_CORE]
        in_maps.append({"imgs": _pack_imgs(shard), "ttoe": ttoe})
    res = run_bass_kernel_spmd(
        nc, in_maps, core_ids=list(range(N_CORES)), trace=trace
    )
    rm = np.concatenate([r["rowmax"] for r in res.results], axis=1)  # [113, B]
    return rm, res


def _toe_host(wn64: np.ndarray) -> np.ndarray:
    """[16*128, 113] stacked width-Toeplitz for exact host rescoring."""
    t = np.zeros((KH, W, SW), dtype=np.float64)
    for j in range(SW):
        t[:, j : j + KW, j] = wn64
    return t.reshape(KH * W, SW)


def _finalize(inputs_np: np.ndarray, wn: np.ndarray, rm: np.ndarray) -> np.ndarray:
    """Host: candidate rows -> exact fp64 rescore -> argmax -> patch scatter."""
    nb = rm.shape[1]
    gm = rm.max(axis=0)  # [nb] device global abs-max per image
    thr = gm * (1.0 - CAND_TOL)
    cb, ci = np.nonzero((rm >= thr[None, :]).T)  # image ids, candidate rows
    n_cand = len(cb)

    wn64 = wn.astype(np.float64)
    toe = _toe_host(wn64)  # [2048, 113]

    # exact scores for each candidate row strip, fp64 via one GEMM per chunk
    row_idx = ci[:, None] + np.arange(KH)[None, :]  # [C, 16]
    scores = np.empty((n_cand, SW), dtype=np.float64)
    chunk = 16384
    for s in range(0, n_cand, chunk):
        e = min(s + chunk, n_cand)
        strips = inputs_np[cb[s:e, None], row_idx[s:e], :]  # [c, 16, 128] f32
        scores[s:e] = strips.reshape(e - s, KH * W).astype(np.float64) @ toe
    scores /= float(N_ELEM)

    # per image: among candidate rows pick max |score|, ties -> lowest flat idx
    flat = ci[:, None].astype(np.int64) * SW + np.arange(SW)[None, :]
    abss = np.abs(scores)
    j_best = np.argmax(abss, axis=1)  # first occurrence within row
    r_abs = abss[np.arange(n_cand), j_best]
    r_val = scores[np.arange(n_cand), j_best]
    r_flat = flat[np.arange(n_cand), j_best]
    # reduce across rows of the same image (first occurrence on exact ties)
    order = np.lexsort((r_flat, -r_abs, cb))  # grouped by image
    cb_o = cb[order]
    first = np.unique(cb_o, return_index=True)[1]
    sel = order[first]
    img_ids = cb[sel]
    best_val = np.zeros(nb, dtype=np.float64)
    best_flat = np.zeros(nb, dtype=np.int64)
    best_abs = np.full(nb, -1.0, dtype=np.float64)
    best_val[img_ids] = r_val[sel]
    best_flat[img_ids] = r_flat[sel]
    best_abs[img_ids] = r_abs[sel]
    assert np.all(best_abs >= 0.0), "some image had no candidate rows"

    rows = (best_flat // SW).astype(np.int64)
    cols = (best_flat % SW).astype(np.int64)
    vals = best_val.astype(np.float32)

    out = np.zeros((nb, H, W), dtype=np.float32)
    patches = vals[:, None, None] * wn[None, :, :]  # [nb, 16, 16] f32
    bidx = np.arange(nb)[:, None, None]
    ridx = rows[:, None, None] + np.arange(KH)[None, :, None]
    cidx = cols[:, None, None] + np.arange(KW)[None, None, :]
    out[bidx, ridx, cidx] = patches
    return out


def kernel(inputs: np.ndarray, w: np.ndarray) -> np.ndarray:
    inputs_np = np.ascontiguousarray(np.asarray(inputs, dtype=np.float32))
    wn = _weights_f32(w)
    rm, _ = _run_device(inputs_np, wn)
    return _finalize(inputs_np, wn, rm)


# revision 6
# speedup vs baseline: 1.0222x; 1.0222x over previous
"""Trainium2 Bass kernel for GTPCA topk_masking layer.

Computation (see reference):
  wn     = w / sqrt(sum(w^2)/n),  n = 128*128
  scores = valid_xcorr2d(inputs, wn) / n          -> (B, 113, 113)
  idx    = argmax |scores| (flat, first occurrence)
  out    = scores[idx] * wn placed as a 16x16 patch at idx, zeros elsewhere

Device strategy (pure data parallel over 8 cores, 512 images each):
  - The 2D correlation runs on the tensor engine as 8 accumulating fp8
    DoubleRow matmuls per 4-image bank.  DoubleRow virtualizes the
    contraction to 256 rows (2 fp8 weights per PE cell, 2 MACs/cycle), so
    kernel columns are processed in pairs (2j, 2j+1): the stationary holds
    the two 128x128 Toeplitz matrices T_{2j} / T_{2j+1} and the moving
    operand holds two column-shifted copies of the image rows (packed by
    the host as separate slabs so no overlapping access patterns are
    needed).  This streams 912 moving columns per image instead of the
    1824 an fp32r kernel needs -> ~2x tensor-engine throughput.
  - Per PSUM bank (4 images) one fused DVE reduce with
    apply_absolute_value produces the per-row abs-max of the score map.
  - Only the per-row abs-max [113, 512] leaves the device.  The host
    keeps candidate rows within CAND_TOL of each image's device global
    max (the gate is sized for fp8 quantization noise), rescores those
    rows exactly in fp64 from the original fp32 inputs, picks the true
    argmax, and scatters smax*wn patches into the output.
"""

import sys

import numpy as np

if "/opt/trn_rl_repo" not in sys.path:
    sys.path.insert(0, "/opt/trn_rl_repo")

import ml_dtypes

F8 = ml_dtypes.float8_e4m3  # TRN FP8_EXP4 (bias 7, max +-240)

N_CORES = 8
B = 4096
H = W = 128
KH = KW = 16
SH = SW = H - KH + 1  # 113
SW_PAD = 113  # streamed score columns per q-pair.  The PE reads the moving
# stream in 16 B/partition beats (8 virtual cols in DoubleRow), so the
# 452-element run rounds up to 456 internally; 113 vs 114 times identically
# but keeps garbage out of the PSUM/reduce.
W2 = 130  # per-slab padded image width (c reads reach col 127; slab ko=1
# col 127 on q-pair 7 is the zero pad).  W2=128 hard-crashes the device
# (NRT_EXEC_UNIT_UNRECOVERABLE) -- keep the 130 padding.
N_ELEM = H * W  # 16384
PER_CORE = B // N_CORES  # 512
GROUP = 16  # images per DMA/compute group
BANK = 4  # images per PSUM bank (456 fp32 <= 512 psum limit)
QP = KW // 2  # 8 q-pairs (DoubleRow processes 2 kernel columns per pass)
CAND_TOL = 8e-2  # candidate-row gate vs device global max (fp8 noise margin)


def _build_nc(n_imgs: int, repeat: int = 1):
    from contextlib import ExitStack

    import concourse.bacc as bacc
    import concourse.mybir as mybir
    import concourse.tile as tile

    f32 = mybir.dt.float32
    f8 = mybir.dt.float8e4

    nc = bacc.Bacc("TRN2", target_bir_lowering=False)
    # imgs[h, quad, ko, c, i] = fp8(img_{4*quad+i}[h, c+ko])  (0 out of range)
    imgs_d = nc.dram_tensor(
        "imgs", [H, n_imgs // BANK, 2, W2, BANK], f8, kind="ExternalInput"
    )
    # ttoe[h, j, ko, m] = fp8(wn[h-m, 2j+ko]) for 0<=h-m<16, m<=112; else 0
    ttoe_d = nc.dram_tensor("ttoe", [H, QP, 2, H], f8, kind="ExternalInput")
    rm_d = nc.dram_tensor("rowmax", [SH, n_imgs], f32, kind="ExternalOutput")

    n_groups = n_imgs // GROUP
    banks_per_group = GROUP // BANK

    with ExitStack() as ctx:
        tc = ctx.enter_context(tile.TileContext(nc))
        consts = ctx.enter_context(tc.tile_pool(name="consts", bufs=1))
        imgp = ctx.enter_context(tc.tile_pool(name="imgp", bufs=3))
        accp = ctx.enter_context(tc.tile_pool(name="accp", bufs=2, space="PSUM"))
        stage = ctx.enter_context(tc.tile_pool(name="stage", bufs=1))

        ttoe_t = consts.tile([H, QP, 2, H], f8)
        nc.sync.dma_start(ttoe_t[:], ttoe_d[:])
        rm_all = stage.tile([SH, n_imgs], f32)

        for _rep in range(repeat):
          for g in range(n_groups):
            img_t = imgp.tile([H, banks_per_group, 2, W2, BANK], f8)
            nc.sync.dma_start(
                img_t[:],
                imgs_d[:, g * banks_per_group : (g + 1) * banks_per_group],
            )

            psums = [
                accp.tile(
                    [H, SW_PAD, BANK], f32, name=f"acc{bk}", tag=f"acc{bk}"
                )
                for bk in range(banks_per_group)
            ]
            for j in range(QP):
                lhsT = ttoe_t[:, j]  # [128, 2, 128]
                for bk in range(banks_per_group):
                    # [128, 2, 114, 4]; (c, i) dims merge to one 456-run
                    rhs = img_t[:, bk, :, 2 * j : 2 * j + SW_PAD, :]
                    nc.tensor.matmul(
                        psums[bk][:],
                        lhsT,
                        rhs,
                        start=(j == 0),
                        stop=(j == QP - 1),
                        perf_mode=mybir.MatmulPerfMode.DoubleRow,
                        skip_group_check=True,
                    )
            for bk in range(banks_per_group):
                base = g * GROUP + bk * BANK
                # [113, 113, 4] -> dims (part, img, c) so X-reduce is over c
                red_in = psums[bk][0:SH, 0:SW, :].transpose([0, 2, 1])
                nc.vector.tensor_reduce(
                    rm_all[:, base : base + BANK],
                    red_in,
                    axis=mybir.AxisListType.X,
                    op=mybir.AluOpType.max,
                    apply_absolute_value=True,
                )

        nc.sync.dma_start(rm_d[:], rm_all[:])

    nc.compile()
    return nc


_NC_CACHE: dict = {}


def _get_nc(n_imgs: int):
    if n_imgs not in _NC_CACHE:
        _NC_CACHE[n_imgs] = _build_nc(n_imgs)
    return _NC_CACHE[n_imgs]


def _weights_f32(w: np.ndarray) -> np.ndarray:
    w32 = np.asarray(w, dtype=np.float32)
    ss = np.sum(w32 * w32, dtype=np.float32)
    denom = np.sqrt(ss / np.float32(N_ELEM))
    return (w32 / denom).astype(np.float32)


def _pack_ttoe(wn: np.ndarray) -> np.ndarray:
    """fp8 stationary: [H, QP, 2, H]; [:, j, ko, m] = T_{2j+ko}[:, m]."""
    wn8 = wn.astype(F8).astype(np.float32)
    ttoe = np.zeros((H, QP, 2, H), dtype=np.float32)
    for m in range(SH):
        for j in range(QP):
            ttoe[m : m + KH, j, 0, m] = wn8[:, 2 * j]
            ttoe[m : m + KH, j, 1, m] = wn8[:, 2 * j + 1]
    return ttoe.astype(F8)


def _pack_imgs(shard: np.ndarray) -> np.ndarray:
    """fp8 moving operand for one core's shard [n, 128, 128] (f32).

    Returns [H, n//4, 2, W2, 4]: two column-shifted fp8 slabs per 4-image
    quad so DoubleRow's ko dim reads non-overlapping memory.
    """
    n = shard.shape[0]
    a8 = np.ascontiguousarray(shard.transpose(1, 0, 2)).astype(F8)  # [H, n, W]
    v = a8.reshape(H, n // BANK, BANK, W).transpose(0, 1, 3, 2)  # [H, q, c, i]
    out = np.zeros((H, n // BANK, 2, W2, BANK), dtype=F8)
    out[:, :, 0, 0:W, :] = v
    out[:, :, 1, 0 : W - 1, :] = v[:, :, 1:, :]
    return out


def _run_device(inputs_np: np.ndarray, wn: np.ndarray, trace: bool = False):
    from concourse.bass_utils import run_bass_kernel_spmd

    nc = _get_nc(PER_CORE)
    ttoe = _pack_ttoe(wn)
    in_maps = []
    for c in range(N_CORES):
        shard = inputs_np[c * PER_CORE : (c + 1) * PER_CORE]
        in_maps.append({"imgs": _pack_imgs(shard), "ttoe": ttoe})
    res = run_bass_kernel_spmd(
        nc, in_maps, core_ids=list(range(N_CORES)), trace=trace
    )
    rm = np.concatenate([r["rowmax"] for r in res.results], axis=1)  # [113, B]
    return rm, res


def _toe_host(wn64: np.ndarray) -> np.ndarray:
    """[16*128, 113] stacked width-Toeplitz for exact host rescoring."""
    t = np.zeros((KH, W, SW), dtype=np.float64)
    for j in range(SW):
        t[:, j : j + KW, j] = wn64
    return t.reshape(KH * W, SW)


def _finalize(inputs_np: np.ndarray, wn: np.ndarray, rm: np.ndarray) -> np.ndarray:
    """Host: candidate rows -> exact fp64 rescore -> argmax -> patch scatter."""
    nb = rm.shape[1]
    gm = rm.max(axis=0)  # [nb] device global abs-max per image
    thr = gm * (1.0 - CAND_TOL)
    cb, ci = np.nonzero((rm >= thr[None, :]).T)  # image ids, candidate rows
    n_cand = len(cb)

    wn64 = wn.astype(np.float64)
    toe = _toe_host(wn64)  # [2048, 113]

    # exact scores for each candidate row strip, fp64 via one GEMM per chunk
    row_idx = ci[:, None] + np.arange(KH)[None, :]  # [C, 16]
    scores = np.empty((n_cand, SW), dtype=np.float64)
    chunk = 16384
    for s in range(0, n_cand, chunk):
        e = min(s + chunk, n_cand)
        strips = inputs_np[cb[s:e, None], row_idx[s:e], :]  # [c, 16, 128] f32
        scores[s:e] = strips.reshape(e - s, KH * W).astype(np.float64) @ toe
    scores /= float(N_ELEM)

    # per image: among candidate rows pick max |score|, ties -> lowest flat idx
    flat = ci[:, None].astype(np.int64) * SW + np.arange(SW)[None, :]
    abss = np.abs(scores)
    j_best = np.argmax(abss, axis=1)  # first occurrence within row
    r_abs = abss[np.arange(n_cand), j_best]
    r_val = scores[np.arange(n_cand), j_best]
    r_flat = flat[np.arange(n_cand), j_best]
    # reduce across rows of the same image (first occurrence on exact ties)
    order = np.lexsort((r_flat, -r_abs, cb))  # grouped by image
    cb_o = cb[order]
    first = np.unique(cb_o, return_index=True)[1]
    sel = order[first]
    img_ids = cb[sel]
    best_val = np.zeros(nb, dtype=np.float64)
    best_flat = np.zeros(nb, dtype=np.int64)
    best_abs = np.full(nb, -1.0, dtype=np.float64)
    best_val[img_ids] = r_val[sel]
    best_flat[img_ids] = r_flat[sel]
    best_abs[img_ids] = r_abs[sel]
    assert np.all(best_abs >= 0.0), "some image had no candidate rows"

    rows = (best_flat // SW).astype(np.int64)
    cols = (best_flat % SW).astype(np.int64)
    vals = best_val.astype(np.float32)

    out = np.zeros((nb, H, W), dtype=np.float32)
    patches = vals[:, None, None] * wn[None, :, :]  # [nb, 16, 16] f32
    bidx = np.arange(nb)[:, None, None]
    ridx = rows[:, None, None] + np.arange(KH)[None, :, None]
    cidx = cols[:, None, None] + np.arange(KW)[None, None, :]
    out[bidx, ridx, cidx] = patches
    return out


def kernel(inputs: np.ndarray, w: np.ndarray) -> np.ndarray:
    inputs_np = np.ascontiguousarray(np.asarray(inputs, dtype=np.float32))
    wn = _weights_f32(w)
    rm, _ = _run_device(inputs_np, wn)
    return _finalize(inputs_np, wn, rm)
